# revision 63
# baseline (speedup 1.0000x reference)
"""Head-parallel multi-head attention on 8 Trainium2 NeuronCores (v2).

Sharding: 2 heads per core (head axis split across 8 cores). Each core
computes its heads' Q/K/V projections (block-diagonal 128x128 weights,
both heads packed on the partition axis), full attention for its 2
heads, and a per-head partial W_o projection over its 128 head-dims.
The host sums the 8 partial outputs (the all-gather + W_o is
algebraically a sum of per-core partial matmuls) and adds b_o.

Structure (~232us vs the ~400us v1 baseline):
  * every matmul is bf16 (fp32 PSUM accumulation) -- same 1 cycle/row
    streaming rate as fp32r but no per-MM 4-byte self-weight-load
    penalty, and FWL kicks in for 128-col weight loads.
  * scores for one k-tile land in a [128, 1024] PSUM tile: head0 ->
    cols 0:512 (bank i), head1 -> cols 512:1024 (bank i+1), issued as
    concurrent 2x-row-tiled matmuls (tile rows 0-63 / 64-127).  ACT
    exp's the whole tile in ONE 1024-wide ACTIVATE (amortizing the
    ~370ns fixed cost), ping-ponged 2-deep so ACT (the bottleneck
    engine, ~155us of exp) never starves.
  * softmax denominator comes from an appended ones-column in the
    packed V operand (PV output row 64).  1/denom = exp(-ln d) on the
    ACT engine (ln+exp share one table set; DVE reciprocal costs
    6.5us per chunk, the custom approx ops are broken on this
    runtime), broadcast across partitions by a K=1 ones-row matmul.
  * the normalize chain is split so it never head-of-line-blocks the
    in-order PE/ACT queues at a phase boundary (a >3.4us PE idle
    lets the HAM clock gate re-throttle the PE array to 1.2 GHz).
  * PSUM = exactly 8 banks: scores ping-pong 2x[128,1024] (4 banks) +
    PV accumulators 2x[128,1024] (4 banks).  The out-projection PSUM
    and batch 1's QKV projections reuse the retired PV accumulator
    tile's banks (subtile WAR deps order them after normalize reads).
  * head 1's normalized output is written to partitions 64:127 by a
    quadrant-aligned cross-bank DVE write, so the out-projection
    contracts both heads in a single K=128 matmul.
  * pipelined phases: phase p runs scores/exp(p) + PV(p-1) +
    normalize/outproj/DMA(p-2); QKV projections are spread across
    earlier phases' slack.
  * output staged to SBUF as bf16 and DMA'd as bf16 (half the HBM
    write traffic); host sums the 8 partials in fp32.
"""

import os
import sys
from contextlib import ExitStack

import numpy as np

for _p in ("/opt/trn_rl_repo", os.path.expanduser("~/.axon_site/_ro/trn_rl_repo")):
    if os.path.isdir(_p) and _p not in sys.path:
        sys.path.append(_p)

import ml_dtypes

import concourse.bass as bass
import concourse.tile as tile
from concourse import mybir
from concourse.bass_utils import run_bass_kernel_spmd

B, S, E, H = 2, 2048, 1024, 16
Dh = E // H           # 64
NCORES = 8
HPC = H // NCORES     # 2 heads per core
PD = HPC * Dh         # 128 pair dims per core
QC = 512              # q-chunk width
NQC = S // QC         # 4
KT = 128              # k-tile rows
NKT = S // KT         # 16
F32 = mybir.dt.float32
BF16 = mybir.dt.bfloat16
EXP = mybir.ActivationFunctionType.Exp
LN = mybir.ActivationFunctionType.Ln
BF = ml_dtypes.bfloat16


def split_multi_waits(nc):
    """Split multi-wait instructions into chained single-wait EventSemaphores.

    The walrus build here accepts at most ONE sync-wait command per
    instruction, while Tile emits several. Rewrite each instruction with
    N>1 waits into (N-1) same-engine EventSemaphore instructions (one
    wait each) followed by the instruction keeping its last wait --
    per-engine program order makes this equivalent.
    """
    n_split = 0
    for f in nc.m.functions:
        for blk in f.blocks:
            insts = list(blk.instructions)
            new = []
            for inst in insts:
                si = inst.sync_info
                waits = list(si.on_wait) if si is not None and si.on_wait else []
                if len(waits) > 1:
                    for j, w in enumerate(waits[:-1]):
                        ev = mybir.InstEventSemaphore(
                            name=f"{inst.name}-wsplit{j}", ins=[], outs=[]
                        )
                        ev.engine = inst.engine
                        ev.sync_info = mybir.SyncInfo(on_wait=[w], on_update=[])
                        nc.register_instruction(ev, overwrite=True)
                        new.append(ev)
                    si.on_wait = waits[-1:]
                    n_split += 1
                new.append(inst)
            blk.instructions = new
    return n_split


def build_program():
    nc = bass.Bass("TRN2", target_bir_lowering=False, debug=False)

    xtb = nc.dram_tensor("xtb", [B, PD, S], BF16, kind="ExternalInput").ap()
    wqkv = nc.dram_tensor("wqkv", [3, PD, PD], BF16, kind="ExternalInput").ap()
    bqk = nc.dram_tensor("bqk", [2, PD, 1], F32, kind="ExternalInput").ap()
    bvb8 = nc.dram_tensor("bvb8", [PD, 8 * PD], F32, kind="ExternalInput").ap()
    wo2 = nc.dram_tensor("wo2", [HPC, Dh, E], BF16, kind="ExternalInput").ap()
    out = nc.dram_tensor("out", [B, S, E], BF16, kind="ExternalOutput").ap()

    with tile.TileContext(nc) as tc, ExitStack() as ctx:
        const = ctx.enter_context(tc.tile_pool(name="const", bufs=1))
        perb = ctx.enter_context(tc.tile_pool(name="perb", bufs=2))
        slabp = ctx.enter_context(tc.tile_pool(name="slab", bufs=32))
        normp = ctx.enter_context(tc.tile_pool(name="norm", bufs=2))
        outp = ctx.enter_context(tc.tile_pool(name="outp", bufs=4))
        ps = ctx.enter_context(tc.tile_pool(name="ps", bufs=2, space="PSUM"))

        # ---- constants (weights first so QKV matmuls start ASAP) ----
        w_sb = []
        for i in range(3):
            w = const.tile([PD, PD], BF16, tag=f"w{i}", name=f"w{i}")
            nc.gpsimd.dma_start(out=w[:], in_=wqkv[i])
            w_sb.append(w)
        xtb_sb = const.tile([PD, B, S], BF16)
        nc.sync.dma_start(out=xtb_sb[:, 0, 0:S // 2], in_=xtb[0, :, 0:S // 2])
        nc.sync.dma_start(out=xtb_sb[:, 0, S // 2:S], in_=xtb[0, :, S // 2:S])
        bq_sb = const.tile([PD, 1], F32, tag="bq")
        nc.sync.dma_start(out=bq_sb[:], in_=bqk[0])
        bk_sb = const.tile([PD, 1], F32, tag="bk")
        nc.sync.dma_start(out=bk_sb[:], in_=bqk[1])
        bvb8_sb = const.tile([PD, 8 * PD], F32, tag="bvb8")
        nc.sync.dma_start(out=bvb8_sb[:], in_=bvb8)
        nc.gpsimd.dma_start(out=xtb_sb[:, 1, :], in_=xtb[1])
        wop_sb = const.tile([PD, E], BF16, tag="wop")
        for h in range(HPC):
            nc.gpsimd.dma_start(out=wop_sb[h * Dh:(h + 1) * Dh, :], in_=wo2[h])
        ones_sb = const.tile([1, Dh], BF16, tag="ones")
        nc.vector.memset(ones_sb[:], 1.0)

        # ---- pipeline state ----
        phases = [(b, c) for b in range(B) for c in range(NQC)]
        NP = len(phases)
        slabs = {}   # phase idx -> list of 16 slab tiles
        oas = {}     # phase idx -> oa PSUM tile [128, 1024] (rows 0:65 used)
        qts = {}     # batch -> qt tile
        kts = {}     # batch -> kt tile
        vaugs = {}   # batch -> vaug tile

        def emit_qkv_alloc(b):
            qt = perb.tile([PD, S], BF16, tag="qt", name=f"qt{b}")
            kt_t = perb.tile([PD, S], BF16, tag="kt", name=f"kt{b}")
            vaug = perb.tile([PD, NKT, HPC, Dh + 1], BF16, tag="vaug",
                             name=f"vaug{b}")
            nc.vector.memset(vaug[:, :, :, Dh], 1.0)
            qts[b], kts[b], vaugs[b] = qt, kt_t, vaug

        def emit_qkv_part(b, part, p=None):
            """One PSUM-slot-sized piece (of 6) of batch b's Q/K/V + drains.

            `p` is the [PD, 2*QC] PSUM tile to route through; defaults to a
            scores-ring slot (batch 0 lead-in).  Batch 1's parts instead
            reuse the retired PV-accumulator tile so they never stall the
            scores/exp ping-pong.
            """
            if p is None:
                p = ps.tile([PD, 2 * QC], F32, tag="scs", name="qkvps")
            kind, g = divmod(part, 2)
            if kind < 2:  # Q or K halves
                dst = qts[b] if kind == 0 else kts[b]
                bias = bq_sb if kind == 0 else bk_sb
                for j in range(2):
                    sl_ = slice((2 * g + j) * QC, (2 * g + j + 1) * QC)
                    nc.tensor.matmul(p[:, j * QC:(j + 1) * QC],
                                     lhsT=w_sb[kind][:],
                                     rhs=xtb_sb[:, b, sl_])
                nc.vector.tensor_scalar_add(
                    dst[:, 2 * g * QC:(2 * g + 2) * QC], p[:], bias[:])
            else:  # V halves
                for i in range(8):
                    st = 8 * g + i
                    nc.tensor.matmul(p[:, i * PD:(i + 1) * PD],
                                     lhsT=xtb_sb[:, b, st * KT:(st + 1) * KT],
                                     rhs=w_sb[2][:])
                nc.vector.tensor_add(
                    vaugs[b][:, 8 * g:8 * (g + 1), :, 0:Dh],
                    p[:].rearrange("p (t h d) -> p t h d", t=8, h=HPC),
                    bvb8_sb[:].rearrange("p (t h d) -> p t h d", t=8, h=HPC),
                )

        def emit_normalize_a(pi):
            """Stage A: denom row (PSUM partition 64) -> DVE copy to
            partition 0.  Emitted at phase top (its input is ready there)."""
            oa = oas[pi]
            dnm = normp.tile([1, 2 * QC], F32, tag="dnm", name="dnm")
            nc.vector.tensor_copy(dnm[:], oa[Dh:Dh + 1, :])
            return dnm

        def emit_normalize_b(pi, dnm):
            """Stage B: 1/d = exp(-ln d) on ACT (ln+exp share one table
            set) -> TensorE K=1 ones-row broadcast across 64 partitions ->
            DVE stage to SBUF -> per-head multiplies.  Head 1's product
            writes partitions 64:128 (quadrant-aligned DVE cross-bank
            write) so the out-projection contracts both heads in one K=128
            matmul.  Emitted a few k-tiles into the next phase so the chain
            never head-of-line-blocks the PE or ACT queues (a ~3.4us PE
            stall here re-throttles the HAM clock gate every phase)."""
            oa = oas[pi]
            lnd = normp.tile([1, 2 * QC], F32, tag="lnd", name="lnd")
            nc.scalar.activation(lnd[:], dnm[:], LN)
            rr = normp.tile([1, 2 * QC], BF16, tag="rr", name="rr")
            nc.scalar.activation(rr[:], lnd[:], EXP, scale=-1.0)
            # broadcast 1/denom across 64 partitions with a col-tiled K=1
            # matmul into the retired oa tile's unused rows 64:127 (instead
            # of stealing a scores-ring slot, which hiccups the exp
            # ping-pong once per phase); row 64 is free once dnm is copied
            oa64 = oa[Dh:Dh + Dh, :]
            for h in range(HPC):
                nc.tensor.matmul(oa64[:, h * QC:(h + 1) * QC],
                                 lhsT=ones_sb[:],
                                 rhs=rr[:, h * QC:(h + 1) * QC],
                                 tile_position=(0, Dh))
            bc = normp.tile([Dh, 2 * QC], F32, tag="bc", name="bc")
            nc.vector.tensor_copy(bc[:], oa64[:])
            otp = normp.tile([PD, QC], BF16, tag="otp", name="otp")
            for h in range(HPC):
                nc.vector.tensor_mul(otp[h * Dh:(h + 1) * Dh, :],
                                     oa[0:Dh, h * QC:(h + 1) * QC],
                                     bc[:, h * QC:(h + 1) * QC])
            return otp

        def emit_outproj_pair(pi, otp, i, outsb, slices=None):
            """Out-projection pair #i (stile i//2, echunk i%2) for phase pi.

            One K=128 matmul (both heads) into the retired oa(pi) PSUM banks
            (slice rotation), then copy to bf16 staging.  Copies alternate
            between DVE and ACT so the copy chain never paces the matmuls.
            """
            oa = oas[pi]
            st, ec = i // 2, i % 2
            esl = slice(ec * QC, (ec + 1) * QC)
            if slices is None:
                sl_ = oa[:, ec * QC:(ec + 1) * QC]
            else:
                sl_ = slices[i % len(slices)]
            nc.tensor.matmul(sl_, lhsT=otp[:, st * KT:(st + 1) * KT],
                             rhs=wop_sb[:, esl])
            if slices is not None and i % 2 == 1:
                # tail only: ACT is idle after the last exp, so splitting
                # the staging copies across DVE+ACT halves the drain chain
                nc.scalar.copy(outsb[:, esl], sl_)
            else:
                nc.vector.tensor_copy(outsb[:, esl], sl_)

        def flush_outproj(pi, kt_idx, state):
            """Interleave outproj work for phase pi-2 at loop position kt_idx."""
            ppi = pi - 2
            if ppi < 0:
                return
            if kt_idx == 2:
                state["ots"] = emit_normalize_b(ppi, state["dnm"])
                return
            if kt_idx < 6 or kt_idx >= 14:
                return
            i = kt_idx - 6
            b2, c2 = phases[ppi]
            st, ec = i // 2, i % 2
            if ec == 0:
                state["outsb"] = outp.tile([KT, E], BF16, tag="outsb",
                                           name="outsb")
            emit_outproj_pair(ppi, state["ots"], i, state["outsb"],
                              slices=state.get("slices"))
            ssl = slice(c2 * QC + st * KT, c2 * QC + (st + 1) * KT)
            esl = slice(ec * QC, (ec + 1) * QC)
            if state.get("slices") is not None:
                # tail: spread over three queues (ACT's DGE is idle by now)
                eng = (nc.sync, nc.gpsimd, nc.scalar)[i % 3]
            else:
                eng = nc.sync if i % 2 == 0 else nc.gpsimd
            # DMA each echunk half as soon as its staging copy lands
            eng.dma_start(out=out[b2, ssl, esl], in_=state["outsb"][:, esl])

        def emit_pv(pi, kt):
            """PV pair for phase pi at k-tile kt."""
            oa = oas[pi]
            sl = slabs[pi][kt]
            for h in range(HPC):
                nc.tensor.matmul(
                    oa[0:Dh + 1, h * QC:(h + 1) * QC],
                    lhsT=vaugs[phases[pi][0]][:, kt, h, :],
                    rhs=sl[:, h * QC:(h + 1) * QC],
                    start=(kt == 0), stop=(kt == NKT - 1),
                )

        state = {}
        emit_qkv_alloc(0)
        emit_qkv_part(0, 0)   # Q first half: covers chunk 0's queries
        emit_qkv_part(0, 2)   # K first half: covers k-tiles 0-7
        emit_qkv_alloc(1)
        for pi, (b, c) in enumerate(phases):
            qt, kt_t = qts[b], kts[b]
            csl = slice(c * QC, (c + 1) * QC)
            slabs[pi] = []
            if pi >= 1:
                oas[pi - 1] = ps.tile([PD, 2 * QC], F32, tag="oa",
                                      name=f"oa{pi - 1}")
            if pi >= 2:
                state["dnm"] = emit_normalize_a(pi - 2)
            for kt in range(NKT):
                scs = ps.tile([PD, 2 * QC], F32, tag="scs", name="scs")
                for h in range(HPC):
                    hsl = slice(Dh * h, Dh * (h + 1))
                    # 2x row tiling: head h occupies PE-array rows 64h..64h+63
                    # (its lhsT/rhs partitions), both heads stream
                    # concurrently, outputs land in different PSUM banks.
                    nc.tensor.matmul(
                        scs[:, h * QC:(h + 1) * QC],
                        lhsT=kt_t[hsl, kt * KT:(kt + 1) * KT],
                        rhs=qt[hsl, csl],
                        tile_position=(Dh * h, 0),
                    )
                sl_t = slabp.tile([PD, 2 * QC], BF16, tag="slab", name="slab")
                nc.scalar.activation(sl_t[:], scs[:], EXP, scale=0.125)
                slabs[pi].append(sl_t)
                if pi >= 1:
                    emit_pv(pi - 1, kt)
                flush_outproj(pi, kt, state)
                # batch 0's remaining QKV parts interleave into phase (0,0)
                # (in-order PE queue: upfront they'd delay the first scores
                # matmul and the first exp by ~7us).  K's second half (part
                # 3) must land before k-tile 8 reads it.
                if pi == 0 and kt in (1, 3, 5, 7):
                    emit_qkv_part(0, {1: 3, 3: 1, 5: 4, 7: 5}[kt])
                # spread batch 1's QKV projections over phases (0,1)-(0,3):
                # phase (0,1) borrows scores-ring slots, (0,2)/(0,3) route
                # through the retired PV accumulator's banks after outproj
                if b == 0 and c >= 1 and kt in (14, 15):
                    emit_qkv_part(1, 2 * (c - 1) + (kt - 14),
                                  p=None if c == 1 else oas[pi - 2])
            if pi >= 1:
                slabs[pi - 1] = None  # release refs (tiles freed by pool reuse)

        # ---- tail: PV for the last phase, then its outproj ----
        last = NP - 1
        oas[last] = ps.tile([PD, 2 * QC], F32, tag="oa", name=f"oa{last}")
        state["dnm"] = emit_normalize_a(last - 1)
        for kt in range(NKT):
            emit_pv(last, kt)
            flush_outproj(NP, kt, state)          # outproj for phase NP-2
        state["dnm"] = emit_normalize_a(last)
        # final outproj has no exp work to hide under: rotate through FOUR
        # retired PSUM slices (both oa parities) so the matmul->copy chain
        # pipelines 4 deep instead of 2
        state["slices"] = [
            oas[last][:, 0:QC], oas[last][:, QC:2 * QC],
            oas[last - 1][:, 0:QC], oas[last - 1][:, QC:2 * QC],
        ]
        for kt in range(2, 14):
            flush_outproj(NP + 1, kt, state)      # outproj for phase NP-1

    from concourse.library_overlay import lower_extended_insts

    lower_extended_insts(nc)
    split_multi_waits(nc)
    return nc


def prep_core_inputs(c, x, Wq, Wk, Wv, bq, bk, bv, Wo):
    h0, h1 = HPC * c, HPC * c + 1
    xT_c = np.ascontiguousarray(
        np.transpose(x[:, :, c * PD:(c + 1) * PD], (0, 2, 1))
    ).astype(BF)
    wqkv = np.zeros((3, PD, PD), np.float32)
    for i, W in enumerate((Wq, Wk, Wv)):
        wqkv[i, :Dh, :Dh] = W[h0]
        wqkv[i, Dh:, Dh:] = W[h1]
    bqk = np.stack([
        np.concatenate([bq[h0], bq[h1]])[:, None],
        np.concatenate([bk[h0], bk[h1]])[:, None],
    ]).astype(np.float32)
    bv_pair = np.concatenate([bv[h0], bv[h1]])          # [128]
    bvb8 = np.tile(bv_pair[None, :], (PD, 8)).astype(np.float32)
    wo2 = np.stack([Wo[h0 * Dh:(h0 + 1) * Dh], Wo[h1 * Dh:(h1 + 1) * Dh]])
    return {
        "xtb": xT_c,
        "wqkv": wqkv.astype(BF),
        "bqk": bqk,
        "bvb8": bvb8,
        "wo2": wo2.astype(BF),
    }


_CACHE = {}


def _get_nc():
    if "nc" not in _CACHE:
        _CACHE["nc"] = build_program()
    return _CACHE["nc"]


def kernel(x, Wq, Wk, Wv, bq, bk, bv, Wo, bo, _trace=False, _trace_kwargs=None):
    x, Wq, Wk, Wv, bq, bk, bv, Wo, bo = (
        np.asarray(a, np.float32) for a in (x, Wq, Wk, Wv, bq, bk, bv, Wo, bo)
    )
    nc = _get_nc()
    in_maps = [
        prep_core_inputs(c, x, Wq, Wk, Wv, bq, bk, bv, Wo) for c in range(NCORES)
    ]
    res = run_bass_kernel_spmd(
        nc, in_maps, list(range(NCORES)), trace=_trace, **(_trace_kwargs or {})
    )
    acc = np.asarray(res.results[0]["out"], np.float32)
    for c in range(1, NCORES):
        acc = acc + np.asarray(res.results[c]["out"], np.float32)
    acc += bo[None, None, :]
    if _trace:
        _CACHE["last_results"] = res
    return acc
